# revision 13
# baseline (speedup 1.0000x reference)
"""BlipAttention (single-head full-C attention) Bass kernel for 8 Trainium2 NeuronCores.

Reference computation (per batch b of 32):
    qkv  = x @ W_qkv + b_qkv          # [1024, 2304]
    q, k, v = split(qkv, 3)           # each [1024, 768]
    S    = (q @ k.T) / sqrt(768)      # [1024, 1024]
    P    = softmax(S, axis=-1)
    out  = (P @ v) @ W_proj + b_proj  # [1024, 768]

Because this is single-head attention over the full C=768 dim, the weight
matrices fold together on the host:

    S   = x (Wq Wk^T) x^T / sqrt(C)  =: x A x^T / sqrt(C)
    out = P x (Wv Wproj) + b_proj    =: P x B + b_proj

so the device never computes q, k or v.  Per batch the device computes

    g^T = A x^T                        (lhsT=wg=A^T chunks, rhs=x^T)   72 MMs
    S^T chunk = g^T-chunk^T x^T        (lhsT=g^T,  rhs=x^T)            96 MMs
    P~^T = exp(scale * S^T)  (bf16)    (no max-subtract: |scores| <~ 5)
    denom = 1^T sum_j P~^T_j           (DVE add tree + one ones-matmul)
    O'^T chunk = sum_j x_j^T P~^T_j    (lhsT=x chunks, rhs=P~^T)       96 MMs
    out_unnorm = O'^T-chunk^T B        (lhsT=O'^T, rhs=wb)             96 MMs

which is ~32% fewer PE cycles than the unfused qkv form.  All matmul operands
are bf16 (fp32 PSUM accumulation); bf16 rounding lands at ~6e-3 max-relative
error vs the fp32 reference (tolerance 2e-2).  Normalization by the softmax
denominator and the b_proj add happen on the host (row scaling commutes with
the right-multiplication by B).  Sharding: data-parallel over B=32 -> 4
batches per core, no collectives.  The reference's setup_inputs always
produces b_qkv == 0; a nonzero b_qkv falls back to an exact host computation.

Schedule notes (v2), from NTFF trace analysis of v1 (314.9us, PE floor 287us):
  - Each DMA trigger instruction costs a fixed ~600ns on the Sync queue, so
    inputs are pre-swizzled on the host so every logical load is ONE trigger
    with long (>=1.5KB) per-partition lines.  Trigger serialization, not HBM
    bandwidth, dominated v1's 13.9us dead head.
  - ~7.2us of runtime preamble (engine barriers, register loads) runs before
    the first DMA trigger can fire.  Warm-up matmuls on memset tiles fill the
    preamble+load window so the PE pstate is fully ramped (0.65->2.4GHz)
    when real matmuls start.
  - Batch 0 / slice 0 of stage A runs 256-wide so the first real matmul only
    needs wg's m=0 chunk (196KB) + a quarter of x^T (393KB).
  - The last two projection blocks DMA straight from PSUM to trim the
    exposed copy->DMA tail chain.
"""

import numpy as np

B = 32
SEQ = 1024
C = 768
NCORES = 8
BL = B // NCORES  # batches per core
P = 128
CK = C // P   # 6 chunks of the C dim
NK = SEQ // P  # 8 chunks of the sequence dim
NQS = 512     # query-slice width (PSUM free-dim limit for fp32)
NSL = SEQ // NQS  # 2 query slices
CS = 384      # cout slice width for proj (768 = 2 x 384)
SCALE = 1.0 / float(np.sqrt(C))
NWARM = 4     # 512-wide warm-up matmuls bridging preamble + cold DMA
NWARM_FINE = 2  # 128-wide warm-ups at the end for a fine-grained handoff

_CACHE = {}


def _build_program():
    import concourse.tile as tile
    import concourse.mybir as mybir
    from concourse import bacc

    F32 = mybir.dt.float32
    F32R = mybir.dt.float32r
    BF16 = mybir.dt.bfloat16
    EXP = mybir.ActivationFunctionType.Exp
    ADD = mybir.AluOpType.add

    nc = bacc.Bacc("TRN2", target_bir_lowering=False, debug=False,
                   num_devices=NCORES)
    # xT swizzled [b, s, p, c, q]: xT_d[b,s,p,c,q] = x[b, s*512+q, c*128+p]
    xT_d = nc.dram_tensor("xT", [BL, NSL, P, CK, NQS], BF16,
                          kind="ExternalInput").ap()
    # xs swizzled [b, p, j, c]: xs_d[b,p,j,c] = x[b, j*128+p, c]
    xs_d = nc.dram_tensor("xs", [BL, P, NK, C], BF16,
                          kind="ExternalInput").ap()
    # wg swizzled [m, p, c, k]: wg_d[m,p,c,k] = wg_host[c*128+p, m*128+k]
    wg_d = nc.dram_tensor("wg", [CK, P, CK, P], BF16,
                          kind="ExternalInput").ap()
    # wb swizzled [p, c, col]: wb_d[p,c,col] = wb_host[c*128+p, col]
    wb_d = nc.dram_tensor("wb", [P, CK, C], BF16, kind="ExternalInput").ap()
    out_d = nc.dram_tensor("out", [BL, SEQ, C], F32, kind="ExternalOutput").ap()
    # [BL*NSL, NQS] so the denominator DMA stays rank-2 on both sides
    # (rank-1 DMA access patterns produce a NEFF the runtime refuses to load)
    dn_d = nc.dram_tensor("dn", [BL * NSL, NQS], F32,
                          kind="ExternalOutput").ap()

    with tile.TileContext(nc) as tc:
        with (
            tc.tile_pool(name="consts", bufs=1) as consts,
            tc.tile_pool(name="xtp", bufs=2) as xtp,
            tc.tile_pool(name="xsp", bufs=2) as xsp,
            tc.tile_pool(name="gtp", bufs=2) as gtp,
            tc.tile_pool(name="ptp", bufs=3) as ptp,
            tc.tile_pool(name="otp", bufs=3) as otp,
            tc.tile_pool(name="dntp", bufs=8) as dntp,
            tc.tile_pool(name="obp", bufs=8) as obp,
            tc.tile_pool(name="smallp", bufs=2) as smallp,
            tc.tile_pool(name="mmp", bufs=7, space="PSUM") as mmp,
            tc.tile_pool(name="dnp", bufs=1, space="PSUM") as dnp,
        ):
            # ---- warm-up: ramp the PE pstate during preamble + cold DMA ----
            wlhs = consts.tile([P, P], BF16, tag="wlhs", name="wlhs")
            wrhs = consts.tile([P, NQS], BF16, tag="wrhs", name="wrhs")
            nc.vector.memset(wlhs[:], 0.03125)
            nc.vector.memset(wrhs[:], 0.03125)
            for _ in range(NWARM):
                ps = mmp.tile([P, NQS], F32, tag="mm", name="ps_w")
                nc.tensor.matmul(ps[:], wlhs[:], wrhs[:], start=True,
                                 stop=True)
            for _ in range(NWARM_FINE):
                ps = mmp.tile([P, NQS], F32, tag="mm", name="ps_w")
                nc.tensor.matmul(ps[:, :P], wlhs[:], wrhs[:, :P], start=True,
                                 stop=True)

            # ---- cold-start loads, finest-critical-path first ----
            xt_tiles = {0: xtp.tile([P, NSL, CK, NQS], BF16, tag="xt",
                                    name="xt")}
            xs_tiles = {0: xsp.tile([P, NK, C], BF16, tag="xs", name="xs")}
            wg = consts.tile([P, CK, C], BF16, tag="wg", name="wg")
            wb = consts.tile([P, CK, C], BF16, tag="wb", name="wb")

            xt0 = xt_tiles[0]
            # batch 0 slice 0 in quarters (enables the 256-wide early start).
            # DMA engines drain trigger FIFOs in order, so non-critical loads
            # (xt q1, xt s1, ...) are triggered strictly after the critical
            # set (xt q0 + wg m=0) and the wg chunks needed right after it.
            nc.sync.dma_start(xt0[:, 0, :, 0:256], xT_d[0, 0, :, :, 0:256])
            nc.sync.dma_start(wg[:, 0, :], wg_d[0])      # m=0 chunk
            for m in range(1, CK):
                nc.sync.dma_start(wg[:, m, :], wg_d[m])
            nc.sync.dma_start(xt0[:, 0, :, 256:512], xT_d[0, 0, :, :, 256:512])
            nc.sync.dma_start(xt0[:, 1, :, :], xT_d[0, 1])
            nc.sync.dma_start(xs_tiles[0][:, :, :], xs_d[0])
            nc.sync.dma_start(wb[:, :, :], wb_d[:, :, :])

            ones_f = consts.tile([P, 1], F32, tag="ones_f", name="ones_f")
            nc.vector.memset(ones_f[:], 1.0)
            ones_t = consts.tile([P, 1], F32R, tag="ones", name="ones")
            nc.scalar.copy(ones_t[:], ones_f[:])

            def load_xt(b):
                t = xt_tiles[b]
                for s in range(NSL):
                    nc.sync.dma_start(t[:, s, :, :], xT_d[b, s])

            def load_xs(b):
                nc.sync.dma_start(xs_tiles[b][:, :, :], xs_d[b])

            for b in range(BL):
                if b not in xt_tiles:
                    xt_tiles[b] = xtp.tile([P, NSL, CK, NQS], BF16, tag="xt",
                                           name="xt")
                    load_xt(b)
                    xs_tiles[b] = xsp.tile([P, NK, C], BF16, tag="xs",
                                           name="xs")
                    load_xs(b)
                xt = xt_tiles[b]
                xs = xs_tiles[b]

                # stage A: g^T = A x^T   (wg chunk (m,c) = A^T block)
                gt = gtp.tile([P, CK, SEQ], BF16, tag="gt", name="gt")
                for s in range(NSL):
                    # batch 0 slice 0: two 256-wide passes so the first
                    # matmul only waits on wg[m=0] + a quarter of x^T
                    qspans = ((0, 256), (256, 512)) if (b == 0 and s == 0) \
                        else ((0, NQS),)
                    for q0, q1 in qspans:
                        qw = q1 - q0
                        for m in range(CK):
                            ps = mmp.tile([P, NQS], F32, tag="mm",
                                          name="ps_a")
                            for c in range(CK):
                                nc.tensor.matmul(
                                    ps[:, :qw],
                                    wg[:, m, c * P:(c + 1) * P],
                                    xt[:, s, c, q0:q1],
                                    start=(c == 0), stop=(c == CK - 1))
                            nc.scalar.copy(
                                gt[:, m, s * NQS + q0:s * NQS + q1],
                                ps[:, :qw])

                for s in range(NSL):
                    nq0 = s * NQS
                    # stage B: S^T chunks + exp
                    pt = ptp.tile([P, NK, NQS], BF16, tag="pt", name="pt")
                    for j in range(NK):
                        ps = mmp.tile([P, NQS], F32, tag="mm", name="ps_s")
                        for c in range(CK):
                            nc.tensor.matmul(
                                ps[:],
                                gt[:, c, j * P:(j + 1) * P],
                                xt[:, s, c, :],
                                start=(c == 0), stop=(c == CK - 1))
                        nc.scalar.activation(pt[:, j, :], ps[:], EXP,
                                             scale=SCALE)
                    # stage C: O'^T chunks
                    ot = otp.tile([P, CK, NQS], BF16, tag="ot", name="ot")
                    for cc in range(CK):
                        ps = mmp.tile([P, NQS], F32, tag="mm", name="ps_o")
                        for j in range(NK):
                            nc.tensor.matmul(
                                ps[:],
                                xs[:, j, cc * P:(cc + 1) * P],
                                pt[:, j, :],
                                start=(j == 0), stop=(j == NK - 1))
                        nc.vector.tensor_copy(ot[:, cc, :], ps[:])

                    def emit_dn():
                        # denominator: DVE add tree over the 8 P~^T chunks,
                        # then a single ones-matmul partition-reduction.
                        t_l1 = []
                        for h in range(4):
                            t = dntp.tile([P, NQS], F32R, tag="dnt",
                                          name="dnt")
                            nc.vector.tensor_tensor(
                                t[:], pt[:, 2 * h, :], pt[:, 2 * h + 1, :],
                                ADD)
                            t_l1.append(t)
                        t_l2 = []
                        for h in range(2):
                            t = dntp.tile([P, NQS], F32R, tag="dnt",
                                          name="dnt")
                            nc.vector.tensor_tensor(
                                t[:], t_l1[2 * h][:], t_l1[2 * h + 1][:], ADD)
                            t_l2.append(t)
                        tsum = dntp.tile([P, NQS], F32R, tag="dnt", name="dnt")
                        nc.vector.tensor_tensor(tsum[:], t_l2[0][:],
                                                t_l2[1][:], ADD)
                        dn = dnp.tile([1, NQS], F32, tag="dn", name="dn")
                        nc.tensor.matmul(dn[:], ones_t[:, :], tsum[:],
                                         start=True, stop=True)
                        rc = smallp.tile([1, NQS], F32, tag="rc", name="rc")
                        nc.vector.tensor_copy(rc[:], dn[:])
                        nc.sync.dma_start(
                            dn_d[b * NSL + s:b * NSL + s + 1, :], rc[:])

                    last_slice = (b == BL - 1 and s == NSL - 1)
                    if last_slice:
                        # keep the reciprocal chain off the kernel tail: the
                        # DVE tree finishes during the stage-C matmuls
                        emit_dn()
                    # stage D: out_unnorm = O' B.  One fused [128,768] DMA
                    # per mi block; the last slice DMAs per-(mi,cs) instead
                    # so the output drains incrementally and the final
                    # transfer isn't queued behind 1MB+ on the DMA engines.
                    for mi in range(NQS // P):
                        ob = obp.tile([P, C], F32, tag="ob", name="ob")
                        for cs in range(2):
                            ps = mmp.tile([P, NQS], F32, tag="mm", name="ps_d")
                            for c in range(CK):
                                nc.tensor.matmul(
                                    ps[:, :CS],
                                    ot[:, c, mi * P:(mi + 1) * P],
                                    wb[:, c, cs * CS:(cs + 1) * CS],
                                    start=(c == 0), stop=(c == CK - 1))
                            nc.vector.tensor_copy(
                                ob[:, cs * CS:(cs + 1) * CS], ps[:, :CS])
                            if last_slice:
                                nc.sync.dma_start(
                                    out_d[b, nq0 + mi * P:nq0 + (mi + 1) * P,
                                          cs * CS:(cs + 1) * CS],
                                    ob[:, cs * CS:(cs + 1) * CS])
                        if not last_slice:
                            nc.sync.dma_start(
                                out_d[b, nq0 + mi * P:nq0 + (mi + 1) * P, :],
                                ob[:])
                    if not last_slice:
                        emit_dn()
    nc.compile()
    return nc


def _get_program():
    if "p" not in _CACHE:
        _CACHE["p"] = _build_program()
    return _CACHE["p"]


def _host_reference(x, W_qkv, b_qkv, W_proj, b_proj):
    out = np.empty((B, SEQ, C), dtype=np.float32)
    for b in range(B):
        qkv = x[b] @ W_qkv + b_qkv
        q, k, v = qkv[:, :C], qkv[:, C:2 * C], qkv[:, 2 * C:]
        s = (q @ k.T) * SCALE
        s -= s.max(axis=-1, keepdims=True)
        np.exp(s, out=s)
        s /= s.sum(axis=-1, keepdims=True)
        out[b] = (s @ v) @ W_proj + b_proj
    return out


def run_sharded(x, W_qkv, b_qkv, b_proj, W_proj, trace=False):
    import ml_dtypes
    from concourse.bass_utils import run_bass_kernel_spmd

    BF = ml_dtypes.bfloat16
    x = np.ascontiguousarray(x, dtype=np.float32)
    W_qkv = np.ascontiguousarray(W_qkv, dtype=np.float32)
    W_proj = np.ascontiguousarray(W_proj, dtype=np.float32)
    b_qkv = np.asarray(b_qkv, dtype=np.float32)
    b_proj = np.asarray(b_proj, dtype=np.float32)

    if np.any(b_qkv):
        # Cannot occur for the reference's setup_inputs (b_qkv is zeros);
        # fall back to an exact host computation for full generality.
        return _host_reference(x, W_qkv, b_qkv, W_proj, b_proj), None

    Wq = W_qkv[:, :C].astype(np.float64)
    Wk = W_qkv[:, C:2 * C].astype(np.float64)
    Wv = W_qkv[:, 2 * C:].astype(np.float64)
    wg_h = (Wk @ Wq.T).astype(np.float32).astype(BF)
    wb_h = (Wv @ W_proj.astype(np.float64)).astype(np.float32).astype(BF)
    # wg_d[m, p, c, k] = wg_h[c*128+p, m*128+k]
    wg = np.ascontiguousarray(
        wg_h.reshape(CK, P, CK, P).transpose(2, 1, 0, 3))
    # wb_d[p, c, col] = wb_h[c*128+p, col]
    wb = np.ascontiguousarray(
        wb_h.reshape(CK, P, C).transpose(1, 0, 2))

    xb = x.astype(BF)
    # xT_d[b, s, p, c, q] = x[b, s*512+q, c*128+p]
    xT = np.ascontiguousarray(
        xb.reshape(B, NSL, NQS, CK, P).transpose(0, 1, 4, 3, 2))
    # xs_d[b, p, j, c] = x[b, j*128+p, c]
    xs = np.ascontiguousarray(
        xb.reshape(B, NK, P, C).transpose(0, 2, 1, 3))

    nc = _get_program()
    in_maps = [
        {"xT": xT[c * BL:(c + 1) * BL], "xs": xs[c * BL:(c + 1) * BL],
         "wg": wg, "wb": wb}
        for c in range(NCORES)
    ]
    res = run_bass_kernel_spmd(nc, in_maps, core_ids=list(range(NCORES)),
                               trace=trace)
    out = np.concatenate([res.results[c]["out"] for c in range(NCORES)],
                         axis=0)
    dn = np.concatenate([res.results[c]["dn"].reshape(BL, SEQ)
                         for c in range(NCORES)], axis=0)
    out = out / dn[:, :, None] + b_proj[None, None, :]
    return out.astype(np.float32), res


def kernel(x, W_qkv, b_qkv, W_proj, b_proj):
    out, _ = run_sharded(x, W_qkv, b_qkv, b_proj, W_proj, trace=False)
    return out


# revision 15
# speedup vs baseline: 1.0070x; 1.0070x over previous
"""BlipAttention (single-head full-C attention) Bass kernel for 8 Trainium2 NeuronCores.

Reference computation (per batch b of 32):
    qkv  = x @ W_qkv + b_qkv          # [1024, 2304]
    q, k, v = split(qkv, 3)           # each [1024, 768]
    S    = (q @ k.T) / sqrt(768)      # [1024, 1024]
    P    = softmax(S, axis=-1)
    out  = (P @ v) @ W_proj + b_proj  # [1024, 768]

Because this is single-head attention over the full C=768 dim, the weight
matrices fold together on the host:

    S   = x (Wq Wk^T) x^T / sqrt(C)  =: x A x^T / sqrt(C)
    out = P x (Wv Wproj) + b_proj    =: P x B + b_proj

so the device never computes q, k or v.  Per batch the device computes

    g^T = A x^T                        (lhsT=wg=A^T chunks, rhs=x^T)   72 MMs
    S^T chunk = g^T-chunk^T x^T        (lhsT=g^T,  rhs=x^T)            96 MMs
    P~^T = exp(scale * S^T)  (bf16)    (no max-subtract: |scores| <~ 5)
    denom = 1^T sum_j P~^T_j           (DVE add tree + one ones-matmul)
    O'^T chunk = sum_j x_j^T P~^T_j    (lhsT=x chunks, rhs=P~^T)       96 MMs
    out_unnorm = O'^T-chunk^T B        (lhsT=O'^T, rhs=wb)             96 MMs

which is ~32% fewer PE cycles than the unfused qkv form.  All matmul operands
are bf16 (fp32 PSUM accumulation); bf16 rounding lands at ~6e-3 max-relative
error vs the fp32 reference (tolerance 2e-2).  Normalization by the softmax
denominator and the b_proj add happen on the host (row scaling commutes with
the right-multiplication by B).  Sharding: data-parallel over B=32 -> 4
batches per core, no collectives.  The reference's setup_inputs always
produces b_qkv == 0; a nonzero b_qkv falls back to an exact host computation.

Schedule notes (v2), from NTFF trace analysis of v1 (314.9us, PE floor 287us):
  - Each DMA trigger instruction costs a fixed ~600ns on the Sync queue, so
    inputs are pre-swizzled on the host so every logical load is ONE trigger
    with long (>=1.5KB) per-partition lines.  Trigger serialization, not HBM
    bandwidth, dominated v1's 13.9us dead head.
  - ~7.2us of runtime preamble (engine barriers, register loads) runs before
    the first DMA trigger can fire.  Warm-up matmuls on memset tiles fill the
    preamble+load window so the PE pstate is fully ramped (0.65->2.4GHz)
    when real matmuls start.
  - Batch 0 / slice 0 of stage A runs 256-wide so the first real matmul only
    needs wg's m=0 chunk (196KB) + a quarter of x^T (393KB).
  - The last two projection blocks DMA straight from PSUM to trim the
    exposed copy->DMA tail chain.
"""

import numpy as np

B = 32
SEQ = 1024
C = 768
NCORES = 8
BL = B // NCORES  # batches per core
P = 128
CK = C // P   # 6 chunks of the C dim
NK = SEQ // P  # 8 chunks of the sequence dim
NQS = 512     # query-slice width (PSUM free-dim limit for fp32)
NSL = SEQ // NQS  # 2 query slices
CS = 384      # cout slice width for proj (768 = 2 x 384)
SCALE = 1.0 / float(np.sqrt(C))
NWARM = 6     # 512-wide warm-up matmuls bridging preamble + cold DMA
NWARM_FINE = 4  # 128-wide warm-ups at the end for a fine-grained handoff

_CACHE = {}


def _build_program():
    import concourse.tile as tile
    import concourse.mybir as mybir
    from concourse import bacc

    F32 = mybir.dt.float32
    F32R = mybir.dt.float32r
    BF16 = mybir.dt.bfloat16
    EXP = mybir.ActivationFunctionType.Exp
    ADD = mybir.AluOpType.add

    nc = bacc.Bacc("TRN2", target_bir_lowering=False, debug=False,
                   num_devices=NCORES)
    # xT swizzled [b, s, p, c, q]: xT_d[b,s,p,c,q] = x[b, s*512+q, c*128+p]
    xT_d = nc.dram_tensor("xT", [BL, NSL, P, CK, NQS], BF16,
                          kind="ExternalInput").ap()
    # xs swizzled [b, p, j, c]: xs_d[b,p,j,c] = x[b, j*128+p, c]
    xs_d = nc.dram_tensor("xs", [BL, P, NK, C], BF16,
                          kind="ExternalInput").ap()
    # wg swizzled [m, p, c, k]: wg_d[m,p,c,k] = wg_host[c*128+p, m*128+k]
    wg_d = nc.dram_tensor("wg", [CK, P, CK, P], BF16,
                          kind="ExternalInput").ap()
    # wb swizzled [p, c, col]: wb_d[p,c,col] = wb_host[c*128+p, col]
    wb_d = nc.dram_tensor("wb", [P, CK, C], BF16, kind="ExternalInput").ap()
    out_d = nc.dram_tensor("out", [BL, SEQ, C], F32, kind="ExternalOutput").ap()
    # [BL*NSL, NQS] so the denominator DMA stays rank-2 on both sides
    # (rank-1 DMA access patterns produce a NEFF the runtime refuses to load)
    dn_d = nc.dram_tensor("dn", [BL * NSL, NQS], F32,
                          kind="ExternalOutput").ap()

    with tile.TileContext(nc) as tc:
        with (
            tc.tile_pool(name="consts", bufs=1) as consts,
            tc.tile_pool(name="xtp", bufs=2) as xtp,
            tc.tile_pool(name="xsp", bufs=2) as xsp,
            tc.tile_pool(name="gtp", bufs=2) as gtp,
            tc.tile_pool(name="ptp", bufs=3) as ptp,
            tc.tile_pool(name="otp", bufs=3) as otp,
            tc.tile_pool(name="dntp", bufs=8) as dntp,
            tc.tile_pool(name="obp", bufs=8) as obp,
            tc.tile_pool(name="smallp", bufs=2) as smallp,
            tc.tile_pool(name="mmp", bufs=7, space="PSUM") as mmp,
            tc.tile_pool(name="dnp", bufs=1, space="PSUM") as dnp,
        ):
            # ---- warm-up: ramp the PE pstate during preamble + cold DMA ----
            wlhs = consts.tile([P, P], BF16, tag="wlhs", name="wlhs")
            wrhs = consts.tile([P, NQS], BF16, tag="wrhs", name="wrhs")
            nc.vector.memset(wlhs[:], 0.03125)
            nc.vector.memset(wrhs[:], 0.03125)
            for _ in range(NWARM):
                ps = mmp.tile([P, NQS], F32, tag="mm", name="ps_w")
                nc.tensor.matmul(ps[:], wlhs[:], wrhs[:], start=True,
                                 stop=True)
            for _ in range(NWARM_FINE):
                ps = mmp.tile([P, NQS], F32, tag="mm", name="ps_w")
                nc.tensor.matmul(ps[:, :P], wlhs[:], wrhs[:, :P], start=True,
                                 stop=True)

            # ---- cold-start loads, finest-critical-path first ----
            xt_tiles = {0: xtp.tile([P, NSL, CK, NQS], BF16, tag="xt",
                                    name="xt")}
            xs_tiles = {0: xsp.tile([P, NK, C], BF16, tag="xs", name="xs")}
            wg = consts.tile([P, CK, C], BF16, tag="wg", name="wg")
            wb = consts.tile([P, CK, C], BF16, tag="wb", name="wb")

            xt0 = xt_tiles[0]
            # batch 0 slice 0 in quarters (enables the 256-wide early start).
            # DMA engines drain trigger FIFOs in order, so non-critical loads
            # (xt q1, xt s1, ...) are triggered strictly after the critical
            # set (xt q0 + wg m=0) and the wg chunks needed right after it.
            nc.sync.dma_start(xt0[:, 0, :, 0:256], xT_d[0, 0, :, :, 0:256])
            nc.sync.dma_start(wg[:, 0, :], wg_d[0])      # m=0 chunk
            nc.sync.dma_start(wg[:, 1, :], wg_d[1])
            nc.sync.dma_start(xt0[:, 0, :, 256:512], xT_d[0, 0, :, :, 256:512])
            for m in range(2, CK):
                nc.sync.dma_start(wg[:, m, :], wg_d[m])
            nc.sync.dma_start(xt0[:, 1, :, :], xT_d[0, 1])
            nc.sync.dma_start(xs_tiles[0][:, :, :], xs_d[0])
            nc.sync.dma_start(wb[:, :, :], wb_d[:, :, :])

            ones_f = consts.tile([P, 1], F32, tag="ones_f", name="ones_f")
            nc.vector.memset(ones_f[:], 1.0)
            ones_t = consts.tile([P, 1], F32R, tag="ones", name="ones")
            nc.scalar.copy(ones_t[:], ones_f[:])

            def load_xt(b):
                t = xt_tiles[b]
                for s in range(NSL):
                    nc.sync.dma_start(t[:, s, :, :], xT_d[b, s])

            def load_xs(b):
                nc.sync.dma_start(xs_tiles[b][:, :, :], xs_d[b])

            for b in range(BL):
                if b not in xt_tiles:
                    xt_tiles[b] = xtp.tile([P, NSL, CK, NQS], BF16, tag="xt",
                                           name="xt")
                    load_xt(b)
                    xs_tiles[b] = xsp.tile([P, NK, C], BF16, tag="xs",
                                           name="xs")
                    load_xs(b)
                xt = xt_tiles[b]
                xs = xs_tiles[b]

                # stage A: g^T = A x^T   (wg chunk (m,c) = A^T block)
                gt = gtp.tile([P, CK, SEQ], BF16, tag="gt", name="gt")
                for s in range(NSL):
                    # batch 0 slice 0: two 256-wide passes so the first
                    # matmul only waits on wg[m=0] + a quarter of x^T
                    qspans = ((0, 256), (256, 512)) if (b == 0 and s == 0) \
                        else ((0, NQS),)
                    for q0, q1 in qspans:
                        qw = q1 - q0
                        for m in range(CK):
                            ps = mmp.tile([P, NQS], F32, tag="mm",
                                          name="ps_a")
                            for c in range(CK):
                                nc.tensor.matmul(
                                    ps[:, :qw],
                                    wg[:, m, c * P:(c + 1) * P],
                                    xt[:, s, c, q0:q1],
                                    start=(c == 0), stop=(c == CK - 1))
                            nc.scalar.copy(
                                gt[:, m, s * NQS + q0:s * NQS + q1],
                                ps[:, :qw])

                for s in range(NSL):
                    nq0 = s * NQS
                    # stage B: S^T chunks + exp
                    pt = ptp.tile([P, NK, NQS], BF16, tag="pt", name="pt")
                    for j in range(NK):
                        ps = mmp.tile([P, NQS], F32, tag="mm", name="ps_s")
                        for c in range(CK):
                            nc.tensor.matmul(
                                ps[:],
                                gt[:, c, j * P:(j + 1) * P],
                                xt[:, s, c, :],
                                start=(c == 0), stop=(c == CK - 1))
                        nc.scalar.activation(pt[:, j, :], ps[:], EXP,
                                             scale=SCALE)
                    # stage C: O'^T chunks
                    ot = otp.tile([P, CK, NQS], BF16, tag="ot", name="ot")
                    for cc in range(CK):
                        ps = mmp.tile([P, NQS], F32, tag="mm", name="ps_o")
                        for j in range(NK):
                            nc.tensor.matmul(
                                ps[:],
                                xs[:, j, cc * P:(cc + 1) * P],
                                pt[:, j, :],
                                start=(j == 0), stop=(j == NK - 1))
                        nc.vector.tensor_copy(ot[:, cc, :], ps[:])

                    def emit_dn():
                        # denominator: DVE add tree over the 8 P~^T chunks,
                        # then a single ones-matmul partition-reduction.
                        t_l1 = []
                        for h in range(4):
                            t = dntp.tile([P, NQS], F32R, tag="dnt",
                                          name="dnt")
                            nc.vector.tensor_tensor(
                                t[:], pt[:, 2 * h, :], pt[:, 2 * h + 1, :],
                                ADD)
                            t_l1.append(t)
                        t_l2 = []
                        for h in range(2):
                            t = dntp.tile([P, NQS], F32R, tag="dnt",
                                          name="dnt")
                            nc.vector.tensor_tensor(
                                t[:], t_l1[2 * h][:], t_l1[2 * h + 1][:], ADD)
                            t_l2.append(t)
                        tsum = dntp.tile([P, NQS], F32R, tag="dnt", name="dnt")
                        nc.vector.tensor_tensor(tsum[:], t_l2[0][:],
                                                t_l2[1][:], ADD)
                        dn = dnp.tile([1, NQS], F32, tag="dn", name="dn")
                        nc.tensor.matmul(dn[:], ones_t[:, :], tsum[:],
                                         start=True, stop=True)
                        rc = smallp.tile([1, NQS], F32, tag="rc", name="rc")
                        nc.vector.tensor_copy(rc[:], dn[:])
                        nc.sync.dma_start(
                            dn_d[b * NSL + s:b * NSL + s + 1, :], rc[:])

                    last_slice = (b == BL - 1 and s == NSL - 1)
                    if last_slice:
                        # keep the reciprocal chain off the kernel tail: the
                        # DVE tree finishes during the stage-C matmuls
                        emit_dn()
                    # stage D: out_unnorm = O' B.  One fused [128,768] DMA
                    # per mi block; the last slice DMAs per-(mi,cs) instead
                    # so the output drains incrementally and the final
                    # transfer isn't queued behind 1MB+ on the DMA engines.
                    for mi in range(NQS // P):
                        ob = obp.tile([P, C], F32, tag="ob", name="ob")
                        for cs in range(2):
                            ps = mmp.tile([P, NQS], F32, tag="mm", name="ps_d")
                            for c in range(CK):
                                nc.tensor.matmul(
                                    ps[:, :CS],
                                    ot[:, c, mi * P:(mi + 1) * P],
                                    wb[:, c, cs * CS:(cs + 1) * CS],
                                    start=(c == 0), stop=(c == CK - 1))
                            nc.vector.tensor_copy(
                                ob[:, cs * CS:(cs + 1) * CS], ps[:, :CS])
                            if last_slice:
                                nc.sync.dma_start(
                                    out_d[b, nq0 + mi * P:nq0 + (mi + 1) * P,
                                          cs * CS:(cs + 1) * CS],
                                    ob[:, cs * CS:(cs + 1) * CS])
                        if not last_slice:
                            nc.sync.dma_start(
                                out_d[b, nq0 + mi * P:nq0 + (mi + 1) * P, :],
                                ob[:])
                    if not last_slice:
                        emit_dn()
    nc.compile()
    return nc


def _get_program():
    if "p" not in _CACHE:
        _CACHE["p"] = _build_program()
    return _CACHE["p"]


def _host_reference(x, W_qkv, b_qkv, W_proj, b_proj):
    out = np.empty((B, SEQ, C), dtype=np.float32)
    for b in range(B):
        qkv = x[b] @ W_qkv + b_qkv
        q, k, v = qkv[:, :C], qkv[:, C:2 * C], qkv[:, 2 * C:]
        s = (q @ k.T) * SCALE
        s -= s.max(axis=-1, keepdims=True)
        np.exp(s, out=s)
        s /= s.sum(axis=-1, keepdims=True)
        out[b] = (s @ v) @ W_proj + b_proj
    return out


def run_sharded(x, W_qkv, b_qkv, b_proj, W_proj, trace=False):
    import ml_dtypes
    from concourse.bass_utils import run_bass_kernel_spmd

    BF = ml_dtypes.bfloat16
    x = np.ascontiguousarray(x, dtype=np.float32)
    W_qkv = np.ascontiguousarray(W_qkv, dtype=np.float32)
    W_proj = np.ascontiguousarray(W_proj, dtype=np.float32)
    b_qkv = np.asarray(b_qkv, dtype=np.float32)
    b_proj = np.asarray(b_proj, dtype=np.float32)

    if np.any(b_qkv):
        # Cannot occur for the reference's setup_inputs (b_qkv is zeros);
        # fall back to an exact host computation for full generality.
        return _host_reference(x, W_qkv, b_qkv, W_proj, b_proj), None

    Wq = W_qkv[:, :C].astype(np.float64)
    Wk = W_qkv[:, C:2 * C].astype(np.float64)
    Wv = W_qkv[:, 2 * C:].astype(np.float64)
    wg_h = (Wk @ Wq.T).astype(np.float32).astype(BF)
    wb_h = (Wv @ W_proj.astype(np.float64)).astype(np.float32).astype(BF)
    # wg_d[m, p, c, k] = wg_h[c*128+p, m*128+k]
    wg = np.ascontiguousarray(
        wg_h.reshape(CK, P, CK, P).transpose(2, 1, 0, 3))
    # wb_d[p, c, col] = wb_h[c*128+p, col]
    wb = np.ascontiguousarray(
        wb_h.reshape(CK, P, C).transpose(1, 0, 2))

    xb = x.astype(BF)
    # xT_d[b, s, p, c, q] = x[b, s*512+q, c*128+p]
    xT = np.ascontiguousarray(
        xb.reshape(B, NSL, NQS, CK, P).transpose(0, 1, 4, 3, 2))
    # xs_d[b, p, j, c] = x[b, j*128+p, c]
    xs = np.ascontiguousarray(
        xb.reshape(B, NK, P, C).transpose(0, 2, 1, 3))

    nc = _get_program()
    in_maps = [
        {"xT": xT[c * BL:(c + 1) * BL], "xs": xs[c * BL:(c + 1) * BL],
         "wg": wg, "wb": wb}
        for c in range(NCORES)
    ]
    res = run_bass_kernel_spmd(nc, in_maps, core_ids=list(range(NCORES)),
                               trace=trace)
    out = np.concatenate([res.results[c]["out"] for c in range(NCORES)],
                         axis=0)
    dn = np.concatenate([res.results[c]["dn"].reshape(BL, SEQ)
                         for c in range(NCORES)], axis=0)
    out = out / dn[:, :, None] + b_proj[None, None, :]
    return out.astype(np.float32), res


def kernel(x, W_qkv, b_qkv, W_proj, b_proj):
    out, _ = run_sharded(x, W_qkv, b_qkv, b_proj, W_proj, trace=False)
    return out
